# revision 1
# baseline (speedup 1.0000x reference)
"""DifferenceAwareAggregator — data-parallel across 8 NeuronCores.

Strategy (per sharding_hint): shard batch dim (B=8192) across the 8 cores,
replicate the small projection weights. Each core computes its B/8=1024
centers end-to-end; results are gathered to the full (B, 512) output.

Algebraic optimizations vs the naive reference:
  concat([h_n, h_n - h_c]) @ W1 == h_n @ (W1_top + W1_bot) - h_c @ W1_bot
which avoids materializing the (B, N, 2H) concat tensor and halves the
big-matmul FLOPs.
"""

import numpy as np
import jax
import jax.numpy as jnp
from functools import partial

N_HEADS = 8
LN_EPS = 1e-5
M = 8  # cores


def _layernorm(x, g, b):
    mu = jnp.mean(x, axis=-1, keepdims=True)
    var = jnp.mean(jnp.square(x - mu), axis=-1, keepdims=True)
    return (x - mu) * jax.lax.rsqrt(var + LN_EPS) * g + b


@partial(jax.pmap, axis_name="i",
         in_axes=(0, 0, 0, None, None, None, None, None, None, None, None,
                  None, None, None, None))
def _shard_fn(h_center, h_neighbors, neighbor_mask,
              W1s, W1b, b1, ln_g, ln_b, Wq, bq, Wk, Wv, bv, Wo, bo):
    B, N, H = h_neighbors.shape
    hd = H // N_HEADS
    # folded difference-aware projection
    z = h_center @ W1b  # (B, H)
    pre = jnp.einsum("bnk,kj->bnj", h_neighbors, W1s) - z[:, None, :] + b1
    hn = jax.nn.gelu(_layernorm(pre, ln_g, ln_b), approximate=False)
    # single-query multi-head attention over neighbors
    Q = (h_center @ Wq + bq).reshape(B, N_HEADS, hd)
    K = jnp.einsum("bnk,kj->bnj", hn, Wk).reshape(B, N, N_HEADS, hd)
    V = (jnp.einsum("bnk,kj->bnj", hn, Wv) + bv).reshape(B, N, N_HEADS, hd)
    scores = jnp.einsum("bhd,bnhd->bhn", Q, K) / np.sqrt(hd)
    scores = jnp.where(neighbor_mask[:, None, :], scores, -jnp.inf)
    attn = jax.nn.softmax(scores, axis=-1)
    ctx = jnp.einsum("bhn,bnhd->bhd", attn, V).reshape(B, H)
    return ctx @ Wo + bo


def kernel(h_center, h_neighbors, W1, b1, ln_g, ln_b, Wq, bq, Wk, bk, Wv, bv,
           Wo, bo, neighbor_mask):
    B, N, H = h_neighbors.shape
    # fold the concat: W1s multiplies h_n, W1b multiplies h_c (subtracted)
    W1 = np.asarray(W1, np.float32)
    W1s = W1[:H] + W1[H:]
    W1b = W1[H:]

    def shard(x):
        return np.asarray(x).reshape(M, B // M, *np.asarray(x).shape[1:])

    # bk drops out of softmax (constant shift per (b,h)) — omitted.
    out = _shard_fn(shard(h_center), shard(h_neighbors), shard(neighbor_mask),
                    W1s, W1b, np.asarray(b1), np.asarray(ln_g),
                    np.asarray(ln_b), np.asarray(Wq), np.asarray(bq),
                    np.asarray(Wk), np.asarray(Wv), np.asarray(bv),
                    np.asarray(Wo), np.asarray(bo))
    return np.asarray(out).reshape(B, H).astype(np.float32)
